# revision 1
# baseline (speedup 1.0000x reference)
"""Trainium2 Bass kernel for nn_DLI_loss_full.

Key algebraic fact: logits[b,j,k] = hw[b,j] + xw[b,k] and the loss is
sum(lse - tgt) over valid groups, so the hw[b,j] term (the whole LSTM
path) cancels exactly:

    per_group[b,j] = log(sum_{k=j+1}^{len_b-1} exp(xw[b,k])) - xw[b,j+1]
    loss = sum(per_group) / sum_b(len_b - 1)

with xw = encoder_output @ w_fc[HID:].  The kernel therefore only
streams encoder_output once (memory-bound), computes xw via
multiply+256-wide reductions, then gets every suffix log-sum-exp with
one hardware suffix-sum scan per 48-wide chunk plus a cross-chunk
combine done as a tiny 128x128 matmul.

Per-core layout: 16 batches x 8 chunks of 48 timesteps = 128 SBUF
partitions, each partition's encoder slice contiguous in DRAM.  All
encoder DMAs ride one HWDGE queue (a single queue sustains ~340 B/ns;
splitting queues loses aggregate bandwidth).
"""

from contextlib import ExitStack

import numpy as np

import concourse.bacc as bacc
import concourse.mybir as mybir
import concourse.tile as tile
from concourse import bass_utils

B, T, D, HID = 128, 384, 256, 256
NCORES = 8
BS = B // NCORES            # 16 batches per core
CH = 8                      # chunks per sequence
L = T // CH                 # 48 timesteps per chunk
P = BS * CH                 # 128 partitions
NP = 8                      # DMA/compute pieces along the free axis
LP = L // NP                # 6 timesteps per piece
F32 = mybir.dt.float32
I32 = mybir.dt.int32
EPS = 1e-30                 # keeps ln() finite on fully-masked tails

# pieces whose multiply runs on gpsimd (plain tensor_tensor + vector
# reduce); late pieces multiply on the faster vector engine so the
# post-DMA tail stays short.
MUL_ON_GPSIMD = (0, 1, 2, 3, 4)

_cache = {}


def _build_nc():
    nc = bacc.Bacc(
        "TRN2", target_bir_lowering=False, debug=False, num_devices=NCORES
    )
    x = nc.dram_tensor("x", [BS, T, D], F32, kind="ExternalInput").ap()
    mk = nc.dram_tensor("mk", [BS, T], I32, kind="ExternalInput").ap()
    wt = nc.dram_tensor("wt", [P, D], F32, kind="ExternalInput").ap()
    um = nc.dram_tensor("um", [P, P], F32, kind="ExternalInput").ap()
    cm = nc.dram_tensor("cm", [P, L], F32, kind="ExternalInput").ap()
    out = nc.dram_tensor("out", [P, 2], F32, kind="ExternalOutput").ap()

    add = mybir.AluOpType.add
    mult = mybir.AluOpType.mult
    bypass = mybir.AluOpType.bypass
    AX = mybir.AxisListType.X
    ACT = mybir.ActivationFunctionType

    with tile.TileContext(nc) as tc, ExitStack() as ctx:
        sp = ctx.enter_context(tc.tile_pool(name="small", bufs=1))
        xp = ctx.enter_context(tc.tile_pool(name="xp", bufs=NP))
        pp = ctx.enter_context(tc.tile_pool(name="psum", bufs=2, space="PSUM"))

        # x-piece loads first, all on the scalar HWDGE queue: it has
        # arbitration priority and sustains ~287 B/ns even while engines
        # read the landing tiles (the sync queue collapses to ~80-185 in
        # the same conditions)
        x_p = x.rearrange("b (c n l) d -> (b c) n (l d)", c=CH, n=NP)
        xts = []
        for i in range(NP):
            xt = xp.tile([P, LP * D], F32, tag="x")
            nc.scalar.dma_start(xt[:], x_p[:, i, :])
            xts.append(xt)

        # small constants ride the gpsimd SWDGE queue so they don't take
        # issue slots from the big stream
        w_sb = sp.tile([P, D], F32)
        nc.gpsimd.dma_start(w_sb[:], wt)
        cm_sb = sp.tile([P, L], F32)
        nc.gpsimd.dma_start(cm_sb[:], cm)
        mi = sp.tile([P, L], I32)
        nc.gpsimd.dma_start(mi[:], mk.rearrange("b (c l) -> (b c) l", c=CH))
        u_sb = sp.tile([P, P], F32)
        nc.gpsimd.dma_start(u_sb[:], um)
        mf = sp.tile([P, L], F32)
        nc.gpsimd.tensor_copy(mf[:], mi[:])

        # warm the Exp activation table while DMA streams
        warm = sp.tile([P, 1], F32)
        nc.scalar.activation(warm[:], cm_sb[:, 0:1], ACT.Exp)

        # replicate w LP times on-chip so the per-piece multiply reads a
        # plain contiguous operand (0-stride broadcast halves DVE rate,
        # and gpsimd cannot read PSUM)
        wrep = sp.tile([P, LP * D], F32)
        nc.vector.tensor_copy(wrep[:, 0:D], w_sb[:])
        nc.vector.tensor_copy(wrep[:, D:2 * D], wrep[:, 0:D])
        nc.vector.tensor_copy(wrep[:, 2 * D:4 * D], wrep[:, 0:2 * D])
        nc.vector.tensor_copy(wrep[:, 4 * D:6 * D], wrep[:, 2 * D:4 * D])
        w3 = wrep[:].rearrange("p (l d) -> p l d", d=D)

        # xw[p, t] = sum_d x[p, t, d] * w[d], piece by piece
        xw = sp.tile([P, L], F32)
        for i in range(NP):
            x3 = xts[i][:].rearrange("p (l d) -> p l d", d=D)
            eng = nc.gpsimd if i in MUL_ON_GPSIMD else nc.vector
            eng.tensor_tensor(x3, x3, w3, mult)
            nc.vector.tensor_reduce(
                xw[:, i * LP:(i + 1) * LP], x3, axis=AX, op=add
            )

        # masked exp, chunk totals, cross-chunk exclusive suffix via matmul
        em = sp.tile([P, L], F32)
        nc.scalar.activation(em[:], xw[:], ACT.Exp)
        # dummy Ln pulls the Ln table load off the serial tail; reading em
        # pins it between the exp above and the real Ln below so the
        # scheduler can't hoist it (which would evict the Exp table)
        lnwarm = sp.tile([P, 1], F32)
        nc.scalar.activation(lnwarm[:], em[:, 0:1], ACT.Ln)
        nc.vector.tensor_mul(em[:], em[:], mf[:])
        tot = sp.tile([P, 1], F32)
        nc.vector.tensor_reduce(tot[:], em[:], axis=AX, op=add)
        aps = pp.tile([P, 1], F32, tag="mm")
        nc.tensor.matmul(aps[:], u_sb[:], tot[:], start=True, stop=True)
        a_sb = sp.tile([P, 1], F32)
        # + EPS seeds every suffix sum, keeping ln() finite on
        # fully-masked tails
        nc.vector.tensor_scalar_add(a_sb[:], aps[:], EPS)

        # within-chunk suffix sums, seeded with the later-chunk total
        ss = sp.tile([P, L], F32)
        nc.vector.tensor_tensor_scan(
            ss[:][:, ::-1], em[:][:, ::-1], em[:][:, ::-1],
            initial=a_sb[:], op0=add, op1=bypass,
        )
        lt = sp.tile([P, L], F32)
        nc.scalar.activation(lt[:], ss[:], ACT.Ln)

        # loss terms: sum over valid groups of (ln(suffix) - xw), and count
        wm = sp.tile([P, L], F32)
        nc.gpsimd.tensor_mul(wm[:], mf[:], cm_sb[:])
        diff = sp.tile([P, L], F32)
        nc.vector.tensor_sub(diff[:], lt[:], xw[:])
        res = sp.tile([P, 2], F32)
        nc.vector.scalar_tensor_tensor(
            out=diff[:], in0=diff[:], scalar=1.0, in1=wm[:],
            op0=bypass, op1=mult, accum_out=res[:, 0:1],
        )
        nc.vector.tensor_reduce(res[:, 1:2], mf[:], axis=AX, op=add)
        nc.sync.dma_start(out, res[:])

    nc.compile()
    return nc


def _host_consts():
    w_idx = np.arange(P)
    um = (
        (w_idx[:, None] // CH == w_idx[None, :] // CH)
        & (w_idx[:, None] % CH > w_idx[None, :] % CH)
    ).astype(np.float32)
    cm = np.ones((P, L), np.float32)
    cm[w_idx % CH == 0, 0] = 0.0
    return um, cm


def kernel(**inputs) -> np.ndarray:
    enc = np.ascontiguousarray(np.asarray(inputs["encoder_output"], np.float32))
    mask = np.ascontiguousarray(np.asarray(inputs["mask"], np.int32))
    w_fc = np.asarray(inputs["w_fc"], np.float32)

    if "nc" not in _cache:
        _cache["nc"] = _build_nc()
    nc = _cache["nc"]

    wt = np.ascontiguousarray(np.broadcast_to(w_fc[HID:], (P, D)), np.float32)
    um, cm = _host_consts()
    in_maps = [
        {
            "x": enc[c * BS:(c + 1) * BS],
            "mk": mask[c * BS:(c + 1) * BS],
            "wt": wt,
            "um": um,
            "cm": cm,
        }
        for c in range(NCORES)
    ]
    res = bass_utils.run_bass_kernel_spmd(
        nc, in_maps, core_ids=list(range(NCORES))
    )
    o = np.stack([r["out"] for r in res.results]).astype(np.float64)
    num = o[:, :, 0].sum()
    den = o[:, :, 1].sum() - B
    return np.asarray(num / den, dtype=np.float32)



# revision 5
# speedup vs baseline: 2.2288x; 2.2288x over previous
"""Trainium2 Bass kernel for nn_DLI_loss_full.

Key algebraic fact: logits[b,j,k] = hw[b,j] + xw[b,k] and the loss is
sum(lse - tgt) over valid groups, so the hw[b,j] term (the whole LSTM
path) cancels exactly:

    per_group[b,j] = log(sum_{k=j+1}^{len_b-1} exp(xw[b,k])) - xw[b,j+1]
    loss = sum(per_group) / sum_b(len_b - 1)

with xw = encoder_output @ w_fc[HID:].

This version feeds x to the TENSOR engine instead of vector/gpsimd
multiply+reduce: the host pre-transposes x per core to
[d_low(128 partitions), l(48), d_half(2), p(128)] in fp8-e4m3 (loss
tolerance is 2e-2; fp8 quantization lands ~1e-4), so each (l, dh)
slice is a ready-made [K=128, M=128] stationary operand and
matmul(xw[:, l], x_slice, w[:, dh]) accumulates xw[p, l] in PSUM.
fp8 also shrinks the HBM stream 4x vs f32 (1.57 MB/core).

Masked (t >= len) tokens' feature vectors are overwritten on the host
with v = -30*w/|w|^2, making exp(xw) ~ e^-30 there: the on-device mask
multiply disappears, and suffix sums stay strictly positive (no EPS
needed).  Per-group weights wm (valid-group indicator) are host-built;
the device computes sum(ln(suffix)*wm), sum(xw*wm) and sum(wm) and the
host combines scalars across cores.
"""

from contextlib import ExitStack

import ml_dtypes
import numpy as np

import concourse.bacc as bacc
import concourse.mybir as mybir
import concourse.tile as tile
from concourse import bass_utils

B, T, D, HID = 128, 384, 256, 256
NCORES = 8
BS = B // NCORES            # 16 batches per core
CH = 8                      # chunks per sequence
L = T // CH                 # 48 timesteps per chunk
P = BS * CH                 # 128 partitions
NP = 4                      # x DMA pieces
LP = L // NP                # 12 l-columns per piece
F32 = mybir.dt.float32

XDT = mybir.dt.float8e4
XNP = ml_dtypes.float8_e4m3
XB = 1                      # bytes per x element

_cache = {}


def _build_nc():
    nc = bacc.Bacc(
        "TRN2", target_bir_lowering=False, debug=False, num_devices=NCORES
    )
    # x transposed: partition = d_low, free = (l, dh, p)
    xt = nc.dram_tensor("xt", [P, L * 2 * P], XDT, kind="ExternalInput").ap()
    wt = nc.dram_tensor("wt", [P, 2], XDT, kind="ExternalInput").ap()
    wm = nc.dram_tensor("wm", [P, L], F32, kind="ExternalInput").ap()
    um = nc.dram_tensor("um", [P, P], F32, kind="ExternalInput").ap()
    out = nc.dram_tensor("out", [P, 3], F32, kind="ExternalOutput").ap()

    add = mybir.AluOpType.add
    mult = mybir.AluOpType.mult
    bypass = mybir.AluOpType.bypass
    AX = mybir.AxisListType.X
    ACT = mybir.ActivationFunctionType

    with tile.TileContext(nc) as tc, ExitStack() as ctx:
        sp = ctx.enter_context(tc.tile_pool(name="small", bufs=1))
        xp = ctx.enter_context(tc.tile_pool(name="xp", bufs=NP))
        pp = ctx.enter_context(tc.tile_pool(name="psum", bufs=2, space="PSUM"))

        # small consts on the sync HWDGE ring (w needed before first MM)
        w_sb = sp.tile([P, 2], XDT)
        nc.sync.dma_start(w_sb[:], wt)
        wm_sb = sp.tile([P, L], F32)
        nc.sync.dma_start(wm_sb[:], wm)
        # um only needed late: gpsimd SWDGE queue keeps the HW rings short
        u_sb = sp.tile([P, P], F32)
        nc.gpsimd.dma_start(u_sb[:], um)

        # x stream on the scalar HWDGE ring, NP pieces so matmuls start early
        xts = []
        for i in range(NP):
            t = xp.tile([P, LP * 2 * P], XDT, tag="x")
            nc.scalar.dma_start(
                t[:], xt[:, i * LP * 2 * P:(i + 1) * LP * 2 * P]
            )
            xts.append(t)

        # warm both activation tables while DMA streams (Ln first: its set
        # may also contain exp, in which case the tail pays zero reloads)
        warm = sp.tile([P, 1], F32)
        nc.scalar.activation(warm[:], wm_sb[:, 0:1], ACT.Ln)
        nc.scalar.activation(warm[:], wm_sb[:, 0:1], ACT.Exp)

        # xw[p, l] = sum_d x[p, l, d] * w[d] on the PE: per (l, dh) the
        # stationary operand is x^T[d_low, p] and the moving operand the
        # matching w column; dh pair accumulates in PSUM
        xw = pp.tile([P, L], F32, tag="xw")
        for i in range(NP):
            for ll in range(LP):
                l = i * LP + ll
                for dh in range(2):
                    nc.tensor.matmul(
                        xw[:, l:l + 1],
                        xts[i][:, (ll * 2 + dh) * P:(ll * 2 + dh + 1) * P],
                        w_sb[:, dh:dh + 1],
                        start=(dh == 0),
                        stop=(dh == 1),
                    )

        res = sp.tile([P, 4], F32)
        # count of valid groups (off the critical path, only needs wm)
        nc.vector.tensor_reduce(res[:, 1:2], wm_sb[:], axis=AX, op=add)

        # sum(xw * wm) per partition: runs on DVE in parallel with exp
        dump0 = sp.tile([P, L], F32)
        nc.vector.scalar_tensor_tensor(
            out=dump0[:], in0=xw[:], scalar=1.0, in1=wm_sb[:],
            op0=bypass, op1=mult, accum_out=res[:, 2:3],
        )

        # masked exp (masking is baked into x) + chunk totals in one op
        em = sp.tile([P, L], F32)
        tot = sp.tile([P, 1], F32)
        nc.scalar.activation(em[:], xw[:], ACT.Exp, accum_out=tot[:])

        # cross-chunk exclusive suffix of chunk totals via matmul
        aps = pp.tile([P, 1], F32, tag="aps")
        nc.tensor.matmul(aps[:], u_sb[:], tot[:], start=True, stop=True)

        # within-chunk suffix sums seeded with the later-chunk total
        ss = sp.tile([P, L], F32)
        nc.vector.tensor_tensor_scan(
            ss[:][:, ::-1], em[:][:, ::-1], em[:][:, ::-1],
            initial=aps[:, 0:1], op0=add, op1=bypass,
        )
        lt = sp.tile([P, L], F32)
        nc.scalar.activation(lt[:], ss[:], ACT.Ln)

        # sum(ln(suffix) * wm); host computes res0 - res2 = loss numerator
        dump1 = sp.tile([P, L], F32)
        nc.vector.scalar_tensor_tensor(
            out=dump1[:], in0=lt[:], scalar=1.0, in1=wm_sb[:],
            op0=bypass, op1=mult, accum_out=res[:, 0:1],
        )
        nc.sync.dma_start(out, res[:, 0:3])

    nc.compile()
    return nc


def _host_consts():
    w_idx = np.arange(P)
    um = (
        (w_idx[:, None] // CH == w_idx[None, :] // CH)
        & (w_idx[:, None] % CH > w_idx[None, :] % CH)
    ).astype(np.float32)
    return um


def _prep_inputs(inputs):
    """FULL inputs -> per-core in_maps for run_bass_kernel_spmd."""
    enc = np.asarray(inputs["encoder_output"], np.float32)
    mask = np.asarray(inputs["mask"], np.int32)
    w_fc = np.asarray(inputs["w_fc"], np.float32)

    w_x = w_fc[HID:]
    # masked tokens get v with v.w = -30 => exp(xw) ~ e^-30 there
    v = (-30.0 / float(np.dot(w_x, w_x))) * w_x
    xm = np.where(mask.astype(bool)[:, :, None], enc, v[None, None, :])
    xq = xm.astype(XNP)
    # [B,T,D] -> [core, b, c, l, dh, dl] -> [core, dl, l, dh, b, c]
    xq = xq.reshape(NCORES, BS, CH, L, 2, P)
    xq = np.ascontiguousarray(xq.transpose(0, 5, 3, 4, 1, 2))
    xq = xq.reshape(NCORES, P, L * 2 * P)

    wq = np.ascontiguousarray(w_x.reshape(2, P).T.astype(XNP))

    # wm[p, l] = group-valid weight: mask, minus the t=0 group
    wm = mask.reshape(NCORES, BS, CH, L).reshape(NCORES, P, L).astype(np.float32)
    wm[:, ::CH, 0] = 0.0  # chunk-0 partitions, l=0 <=> t=0: not a group
    um = _host_consts()

    return [
        {"xt": xq[c], "wt": wq, "wm": wm[c], "um": um}
        for c in range(NCORES)
    ]


def kernel(**inputs) -> np.ndarray:
    if "nc" not in _cache:
        _cache["nc"] = _build_nc()
    nc = _cache["nc"]

    in_maps = _prep_inputs(inputs)
    res = bass_utils.run_bass_kernel_spmd(
        nc, in_maps, core_ids=list(range(NCORES))
    )
    o = np.stack([r["out"] for r in res.results]).astype(np.float64)
    num = (o[:, :, 0] - o[:, :, 2]).sum()
    den = o[:, :, 1].sum()
    return np.asarray(num / den, dtype=np.float32)
